# revision 2
# baseline (speedup 1.0000x reference)
"""BernoulliRBF retrieval kernel for 8 trn2 NeuronCores (ref-stationary).

Cores 0-3 hold pos shards, 4-7 neg shards (8192 refs each). Per core:
    psum[m, n] = 2 xs_n . ys_m      (fp16 matmuls, K=256 in 2 chunks; each
                                     128-ref m-tile -> two [128,1024] psum
                                     tiles, 4-deep rotation)
    E[m, n] = exp(psum + wk_m)      wk_m = -|ys_m|^2 + K. Most m-tiles:
                                     ACT from PSUM with per-partition bias
                                     (the bias pass costs nothing and no
                                     separate PSUM->SBUF hop is needed).
                                     Every 7th m-tile: DVE Schraudolph
                                     (y = t*128/ln2 + bits-bias; max(y,0.5)
                                     converted to int16 == bf16 exp bits)
                                     to offload ~15% of ScalarE work.
    ladder: bf16 pair-adds on DVE (2x mode) fold 8 m-tiles -> 1 tile;
    out: 9 ladder tops [128, 2048] bf16 -> host sums in float64.

Per-query shift C_n is unnecessary: with a global per-set shift K chosen so
max(t)+K <= ~20, exp never overflows fp32/bf16, and underflow (terms e^-80
below a query's max) is harmless to the sum. Engine busy ~= tensor 122us,
scalar 116us, vector 100us -> ~147us wall (baseline 188us).

host: log_count = log(sum E) - K - |xs_n|^2 - wb;
      log_p_x = log_pos - logaddexp(log_pos, log_neg).
"""
import os
import numpy as np
from contextlib import ExitStack

N, M, D = 2048, 32768, 256
NCORES = 8
CORES_PER_SET = 4
SHARD = M // CORES_PER_SET      # 8192 refs per core
MT = SHARD // 128               # 64 m-tiles per core
LADDER_SIZES = [8] * 7 + [4, 4]  # m-tiles folded per output tile
NLAD = len(LADDER_SIZES)        # 9 output tiles per core
SAMPLE_STRIDE = 64
K_MARGIN = 25.0                 # sampled-max safety margin
K_TARGET = 20.0                 # target max exponent after shift
SCH_SCALE = 128.0 / np.log(2.0)  # Schraudolph bf16-bits scale
SCH_BIAS = 127.0 * 128.0 - 7.41  # exponent bias minus mid-range correction

LAST_EXEC_NS = None

_cache = {}


def _build():
    import concourse.tile as tile
    from concourse import bacc, mybir

    F32, F16, BF16 = mybir.dt.float32, mybir.dt.float16, mybir.dt.bfloat16
    EXP = mybir.ActivationFunctionType.Exp

    I16 = mybir.dt.int16

    nc = bacc.Bacc("TRN2", target_bir_lowering=False, debug=False)
    A = nc.dram_tensor("A", [2, 128, N], F16, kind="ExternalInput").ap()
    B = nc.dram_tensor("B", [2, 128, SHARD], F16, kind="ExternalInput").ap()
    WK = nc.dram_tensor("WK", [128, MT], F32, kind="ExternalInput").ap()
    WK2 = nc.dram_tensor("WK2", [128, MT], F32, kind="ExternalInput").ap()
    EO = nc.dram_tensor("EO", [NLAD, 128, N], BF16, kind="ExternalOutput").ap()

    with tile.TileContext(nc) as tc:
        with ExitStack() as ctx:
            sing = ctx.enter_context(tc.tile_pool(name="sing", bufs=1))
            psums = ctx.enter_context(tc.tile_pool(name="psum", bufs=1, space="PSUM"))

            dvexp = os.environ.get("BASS_V3_DVEXP", "1") not in ("", "0")
            # m-tiles whose exp runs on DVE (Schraudolph bits trick)
            dve_tiles = set(range(3, MT, 7)) if dvexp else set()

            wk_sb = sing.tile([128, MT], F32)
            wk2_sb = sing.tile([128, MT], F32)
            a_sb = sing.tile([128, 2, N], F16)
            b_sb = sing.tile([128, 2, SHARD], F16)
            # prologue-critical first: m-tile 0 of B, first query block of A
            for d in range(2):
                nc.gpsimd.dma_start(out=b_sb[:, d, 0:128], in_=B[d][:, 0:128])
            for d in range(2):
                nc.sync.dma_start(out=a_sb[:, d, 0:1024], in_=A[d][:, 0:1024])
            nc.gpsimd.dma_start(out=wk_sb[:], in_=WK[:])
            if dvexp:
                nc.gpsimd.dma_start(out=wk2_sb[:], in_=WK2[:])
            for d in range(2):
                nc.sync.dma_start(out=a_sb[:, d, 1024:2048], in_=A[d][:, 1024:2048])
            for d in range(2):
                nc.gpsimd.dma_start(out=b_sb[:, d, 128:1024], in_=B[d][:, 128:1024])
            for c0 in range(1024, SHARD, 1024):
                eng = nc.gpsimd if (c0 // 1024) % 2 else nc.sync
                for d in range(2):
                    eng.dma_start(out=b_sb[:, d, c0:c0 + 1024],
                                  in_=B[d][:, c0:c0 + 1024])

            halfact = os.environ.get("BASS_V3_HALFACT", "1") not in ("", "0")
            nptile = 4 if halfact else 2
            ptiles = []
            for i in range(nptile):
                pt = psums.tile([128, 4096 // nptile], F32, name=f"pt{i}")
                ptiles.append(pt)
            ebufs = []
            for i in range(4):
                eb = sing.tile([128, N], BF16, name=f"eb{i}")
                ebufs.append(eb)
            pbufs = []
            for i in range(2):
                pb = sing.tile([128, N], BF16, name=f"pb{i}")
                pbufs.append(pb)
            qbufs = []
            for i in range(2):
                qb = sing.tile([128, N], BF16, name=f"qb{i}")
                qbufs.append(qb)
            tops = []
            for i in range(2):
                tp = sing.tile([128, N], BF16, name=f"top{i}")
                tops.append(tp)
            ybufs = []
            for i in range(2):
                yb = sing.tile([128, 1024], F32, name=f"yb{i}")
                ybufs.append(yb)
            SCH_S = SCH_SCALE

            mt_base = 0
            for L, lsz in enumerate(LADDER_SIZES):
                for j in range(lsz):
                    mt = mt_base + j
                    msl = slice(mt * 128, (mt + 1) * 128)
                    if halfact:
                        douter = os.environ.get("BASS_V3_DOUTER", "") not in ("", "0")
                        # two 1024-col psum tiles per m-tile; 4-deep rotation
                        for h in range(2):
                            p = ptiles[(mt * 2 + h) % 4]
                            cd = ([(c, d) for d in range(2) for c in range(2)]
                                  if douter else
                                  [(c, d) for c in range(2) for d in range(2)])
                            for c, d in cd:
                                nc.tensor.matmul(
                                    p[:, c * 512:(c + 1) * 512],
                                    b_sb[:, d, msl],
                                    a_sb[:, d, (h * 2 + c) * 512:
                                               (h * 2 + c + 1) * 512],
                                    start=(d == 0),
                                    stop=(d == 1),
                                    skip_group_check=douter,
                                )
                            if mt in dve_tiles:
                                yb = ybufs[h]
                                nc.vector.tensor_scalar(
                                    out=yb[:], in0=p[:],
                                    scalar1=SCH_S,
                                    scalar2=wk2_sb[:, mt:mt + 1],
                                    op0=mybir.AluOpType.mult,
                                    op1=mybir.AluOpType.add,
                                )
                                nc.vector.tensor_scalar(
                                    out=ebufs[j % 4][
                                        :, h * 1024:(h + 1) * 1024
                                    ].bitcast(I16),
                                    in0=yb[:], scalar1=0.5, scalar2=None,
                                    op0=mybir.AluOpType.max,
                                )
                            else:
                                nc.scalar.activation(
                                    out=ebufs[j % 4][:, h * 1024:(h + 1) * 1024],
                                    in_=p[:], func=EXP,
                                    bias=wk_sb[:, mt:mt + 1], scale=1.0,
                                )
                    else:
                        p = ptiles[mt % 2]
                        if os.environ.get("BASS_V3_DOUTER", "") not in ("", "0"):
                            for d in range(2):
                                for c in range(4):
                                    nc.tensor.matmul(
                                        p[:, c * 512:(c + 1) * 512],
                                        b_sb[:, d, msl],
                                        a_sb[:, d, c * 512:(c + 1) * 512],
                                        start=(d == 0),
                                        stop=(d == 1),
                                        skip_group_check=True,
                                    )
                        else:
                            for c in range(4):
                                for d in range(2):
                                    nc.tensor.matmul(
                                        p[:, c * 512:(c + 1) * 512],
                                        b_sb[:, d, msl],
                                        a_sb[:, d, c * 512:(c + 1) * 512],
                                        start=(d == 0),
                                        stop=(d == 1),
                                    )
                        nc.scalar.activation(
                            out=ebufs[j % 4][:], in_=p[:], func=EXP,
                            bias=wk_sb[:, mt:mt + 1], scale=1.0,
                        )
                    if j % 2 == 1:
                        nc.vector.tensor_add(
                            pbufs[(j // 2) % 2][:],
                            ebufs[(j - 1) % 4][:], ebufs[j % 4][:],
                        )
                    if lsz == 8:
                        if j == 3:
                            nc.vector.tensor_add(qbufs[0][:], pbufs[0][:],
                                                 pbufs[1][:])
                        if j == 7:
                            nc.vector.tensor_add(qbufs[1][:], pbufs[0][:],
                                                 pbufs[1][:])
                            nc.vector.tensor_add(tops[L % 2][:], qbufs[0][:],
                                                 qbufs[1][:])
                    elif lsz == 4:
                        if j == 3:
                            nc.vector.tensor_add(tops[L % 2][:], pbufs[0][:],
                                                 pbufs[1][:])
                mt_base += lsz
                nc.sync.dma_start(out=EO[L], in_=tops[L % 2][:])

    nc.compile()
    return nc


def _prep_set(x, data, scale):
    """Host-side prep for one reference set."""
    xs = (x * scale[None, :]).astype(np.float32)
    ys = (data * scale[None, :]).astype(np.float32)
    A = np.ascontiguousarray((2.0 * xs).T).reshape(2, 128, N).astype(np.float16)
    BT = np.ascontiguousarray(ys.T).reshape(2, 128, M).astype(np.float16)
    ysq = (ys.astype(np.float64) ** 2).sum(axis=1)          # [M]
    # global shift: K = K_TARGET - (sampled max + margin)
    samp = ys[::SAMPLE_STRIDE]
    t_s = 2.0 * (xs @ samp.T) - ysq[::SAMPLE_STRIDE][None, :].astype(np.float32)
    kshift = np.float32(K_TARGET - (float(t_s.max()) + K_MARGIN))
    # wk[m] = -|ys_m|^2 + K, laid out [128, MT] per core later
    wk = (-ysq + np.float64(kshift)).astype(np.float32)      # [M]
    xsq = (xs.astype(np.float64) ** 2).sum(axis=1)           # [N]
    return A, BT, wk, float(kshift), xsq


def kernel(x, data_pos, data_neg, scales_pos, scales_neg, weight_bias):
    global LAST_EXEC_NS
    from concourse.bass_utils import run_bass_kernel_spmd

    x = np.asarray(x, dtype=np.float32)
    data_pos = np.asarray(data_pos, dtype=np.float32)
    data_neg = np.asarray(data_neg, dtype=np.float32)
    scales_pos = np.asarray(scales_pos, dtype=np.float32)
    scales_neg = np.asarray(scales_neg, dtype=np.float32)
    weight_bias = np.asarray(weight_bias, dtype=np.float32)

    if "nc" not in _cache:
        _cache["nc"] = _build()
    nc = _cache["nc"]

    prep_p = _prep_set(x, data_pos, scales_pos)
    prep_n = _prep_set(x, data_neg, scales_neg)

    in_maps = []
    for core in range(NCORES):
        A_, BT_, wk_, _, _ = prep_p if core < CORES_PER_SET else prep_n
        sh = core % CORES_PER_SET
        sl = slice(sh * SHARD, (sh + 1) * SHARD)
        wk_core = wk_[sl].reshape(MT, 128).T                 # [128, MT]
        in_maps.append({
            "A": A_,
            "B": np.ascontiguousarray(BT_[:, :, sl]),
            "WK": np.ascontiguousarray(wk_core),
            "WK2": np.ascontiguousarray(
                (wk_core.astype(np.float64) * SCH_SCALE + SCH_BIAS)
            ).astype(np.float32),
        })

    trace = os.environ.get("BASS_TRACE", "") not in ("", "0")
    res = run_bass_kernel_spmd(nc, in_maps, list(range(NCORES)), trace=trace)
    LAST_EXEC_NS = res.exec_time_ns

    def reduce_set(cores, kshift, xsq, wb):
        tot = np.zeros(N)
        for core in cores:
            eo = np.asarray(res.results[core]["EO"]).astype(np.float64)
            tot += eo.sum(axis=(0, 1))
        return np.log(tot) - kshift - xsq - float(wb)

    log_pos = reduce_set(range(CORES_PER_SET), prep_p[3], prep_p[4],
                         weight_bias[0])
    log_neg = reduce_set(range(CORES_PER_SET, NCORES), prep_n[3], prep_n[4],
                         weight_bias[1])
    log_weight = np.logaddexp(log_pos, log_neg)
    log_p_x = log_pos - log_weight
    return (log_p_x.astype(np.float32), log_weight.astype(np.float32))
